# revision 31
# baseline (speedup 1.0000x reference)
"""Trainium2 Bass kernel for the LSTM decoder (nn_Decoder).

Math (reference):
    u0 = x @ W_u0.T + b_u0
    xi0 = [z, u0, enc]                       # CAT = 64 + 128 + 256 = 448
    h0 = xi0 @ W_h1.T + b_h1 ; c0 = xi0 @ W_h2.T + b_h2
    for t in range(T):
        xi = [z, y_{t-1}, enc]               # y_{-1} = u0, y_t = h_t
        gates = xi @ W_ih.T + h @ W_hh.T + b_ih + b_hh
        i,f,g,o = split(gates); c = sig(f)*c + sig(i)*tanh(g); h = sig(o)*tanh(c)
        y_t = h

Key restructuring:
  * z/enc are time-invariant -> their gate contribution gc = z@Wz.T + enc@We.T + b
    is computed once; per-step matmul is only K=128 (h) after merging
    Wc = W_ih[:, y-cols] + W_hh (valid for t >= 1 where y == h).
  * Layout: hidden (128) on partitions, batch on the free dim. h_t is produced
    directly in the rhs layout the next matmul needs -> zero transposes in loop.
  * Gates reordered [g, f, i, o]: tanh(g) issues right after the first
    recurrent matmul; sigmoid[f|i] spans 2 PSUM banks; sigma(o) is off the
    critical path. [sig_f|sig_i] * [c|tanh_g] is one wide DVE multiply.
  * Loop tensors (sig/tanh outputs, c, h, products) in bf16: DVE tensor ops
    run in 2x packed mode, halving the recurrence-cycle latency that bounded
    the fp32 version. Loop matmuls (gc-injection id-matmuls + recurrent
    h @ Wc) also bf16 (1 cyc/row); precompute matmuls stay fp32r.
  * gc injected into PSUM by identity-matmuls prefetched one step ahead;
    inputs packed into few DMA transfers; cross-engine buffers ping-pong by
    step parity. y is stored/DMA'd bf16 and widened to fp32 on host.
  * Data parallel over 8 cores (batch 8192 -> 1024/core); weights replicated.
"""

import sys

sys.path.insert(0, "/opt/trn_rl_repo")

import ml_dtypes
import numpy as np

import concourse.bass as bass  # noqa: F401  (bass must import before bacc)
import concourse.mybir as mybir
import concourse.tile as tile
from concourse import bacc
from concourse.bass_utils import run_bass_kernel_spmd

N_CORES = 8
BS, IN, HID, LAT, OUT = 8192, 48, 256, 64, 128
B = BS // N_CORES  # 1024 batch rows per core
CH = 512           # batch chunk (one PSUM bank per gate tile)
NCH = B // CH      # 2 chunks
F32 = mybir.dt.float32
F32R = mybir.dt.float32r
BF16 = mybir.dt.bfloat16
AF = mybir.ActivationFunctionType

# gate reorder: torch order i,f,g,o -> g,f,i,o (g first: tanh(g) can issue
# right after the first recurrent matmul; o last: sigma(o) off critical path)
GATE_PERM = np.r_[256:384, 128:256, 0:128, 384:512]

# fp32r packs (precompute path)
PK64_SPEC = [("zT", 1024), ("WzT", 512), ("Wh1z", 128), ("Wh2z", 128)]
PKE0_SPEC = [("eT0", 1024), ("WeT0", 512)]
PKE1_SPEC = [("eT1", 1024), ("WeT1", 512)]
PK48_SPEC = [("xT", 1024), ("Wu0T", 128)]
PK128B_SPEC = [("Wh1y", 128), ("Wh1e0", 128), ("Wh1e1", 128),
               ("Wh2y", 128), ("Wh2e0", 128), ("Wh2e1", 128)]
# bf16 pack (scan-loop weights)
PKBF_SPEC = [("WcT", 512), ("WihyT", 512), ("WhhT", 512), ("idm", 128)]
PK64_W = sum(w for _, w in PK64_SPEC)
PKE0_W = sum(w for _, w in PKE0_SPEC)
PKE1_W = sum(w for _, w in PKE1_SPEC)
PK48_W = sum(w for _, w in PK48_SPEC)
PK128B_W = sum(w for _, w in PK128B_SPEC)
PKBF_W = sum(w for _, w in PKBF_SPEC)

_PROGRAM_CACHE: dict = {}


def build_program(n_steps: int, T_out: int, loops: int = 1):
    """Emit the Bass/Tile program. Runs `n_steps` LSTM steps, writing step t's
    output to y[t % T_out]. With loops > 1 (timing runs only) the whole scan
    is additionally wrapped in a hardware For_i loop, with every step taking
    the steady-state path (h0/c0 as the carry-in) so one launch executes
    loops * n_steps steps with a small program."""
    nc = bacc.Bacc("TRN2", target_bir_lowering=False, debug=False)

    # packed inputs (one DMA each): see _prep_maps for column layouts
    d_pk64 = nc.dram_tensor("pk64", [LAT, PK64_W], F32R, kind="ExternalInput")
    d_pke0 = nc.dram_tensor("pke0", [128, PKE0_W], F32R, kind="ExternalInput")
    d_pke1 = nc.dram_tensor("pke1", [128, PKE1_W], F32R, kind="ExternalInput")
    d_pk48 = nc.dram_tensor("pk48", [IN, PK48_W], F32R, kind="ExternalInput")
    d_pk128b = nc.dram_tensor("pk128b", [128, PK128B_W], F32R, kind="ExternalInput")
    d_pkbf = nc.dram_tensor("pkbf", [128, PKBF_W], BF16, kind="ExternalInput")
    d_bias = nc.dram_tensor("bias7", [128, 7], F32, kind="ExternalInput")
    d_y = nc.dram_tensor("y", [T_out, OUT, B], BF16, kind="ExternalOutput")
    y_ap = d_y[:]

    with tile.TileContext(nc) as tc:
        with (
            tc.tile_pool(name="const", bufs=1) as cpool,
            tc.tile_pool(name="state", bufs=1) as spool,
            tc.tile_pool(name="psum", bufs=1, space="PSUM") as ppool,
        ):
            # ---- load packed inputs, split across both HWDGE rings
            # (sync/SP and scalar/ACT) so the two streams overlap; ordered
            # so the gc (z/enc) and u0 (x) chains start ASAP ----
            def load(dram, shape, tag, dt=F32, eng=None):
                t = cpool.tile(shape, dt, tag=tag, name=tag)
                (eng or nc.sync).dma_start(out=t[:], in_=dram[:])
                return t

            pk64 = load(d_pk64, [LAT, PK64_W], "pk64", F32R)
            pke0 = load(d_pke0, [128, PKE0_W], "pke0", F32R, eng=nc.scalar)
            pke1 = load(d_pke1, [128, PKE1_W], "pke1", F32R)
            bias = load(d_bias, [128, 7], "bias", eng=nc.scalar)
            pk48 = load(d_pk48, [IN, PK48_W], "pk48", F32R, eng=nc.scalar)
            pkbf = load(d_pkbf, [128, PKBF_W], "pkbf", BF16)
            pk128b = load(d_pk128b, [128, PK128B_W], "pk128b", F32R,
                          eng=nc.scalar)

            def cols(t, specs):
                out, o = {}, 0
                for nm, w in specs:
                    out[nm] = t[:, o:o + w]
                    o += w
                return out

            c64 = cols(pk64, PK64_SPEC)
            ce0 = cols(pke0, PKE0_SPEC)
            ce1 = cols(pke1, PKE1_SPEC)
            c48 = cols(pk48, PK48_SPEC)
            cb = cols(pk128b, PK128B_SPEC)
            cf = cols(pkbf, PKBF_SPEC)
            zT, WzT = c64["zT"], c64["WzT"]
            Wh1z, Wh2z = c64["Wh1z"], c64["Wh2z"]
            eT0, eT1 = ce0["eT0"], ce1["eT1"]
            WeT0, WeT1 = ce0["WeT0"], ce1["WeT1"]
            xT, Wu0T = c48["xT"], c48["Wu0T"]
            WcT, WihyT, WhhT = cf["WcT"], cf["WihyT"], cf["WhhT"]
            idm = cf["idm"]
            Wh = {"Wh1": [Wh1z, cb["Wh1y"], cb["Wh1e0"], cb["Wh1e1"]],
                  "Wh2": [Wh2z, cb["Wh2y"], cb["Wh2e0"], cb["Wh2e1"]]}

            # ---- persistent loop state ----
            gc = spool.tile([128, 4 * B], BF16, tag="gc", name="gc")  # [gate_tile, batch]
            u0 = spool.tile([128, B], F32R, tag="u0", name="u0")
            u0b = spool.tile([128, B], BF16, tag="u0b", name="u0b")
            h0 = spool.tile([128, B], BF16, tag="h0", name="h0")
            prod = [spool.tile([128, 2 * CH], BF16, tag=f"prod{c}", name=f"prod{c}") for c in range(NCH)]
            sfi = [[spool.tile([128, 2 * CH], BF16, tag=f"sfi{c}{p}", name=f"sfi{c}{p}")
                    for p in range(2)] for c in range(NCH)]
            sob = [[spool.tile([128, CH], BF16, tag=f"so{c}{p}", name=f"so{c}{p}")
                    for p in range(2)] for c in range(NCH)]
            tcell = [[spool.tile([128, CH], BF16, tag=f"tc{c}{p}", name=f"tc{c}{p}")
                      for p in range(2)] for c in range(NCH)]
            # pair[c][p] = [c_cell | tanh_g] ; h ping-pong per chunk
            pair = [[spool.tile([128, 2 * CH], BF16, tag=f"pair{c}{p}", name=f"pair{c}{p}") for p in range(2)]
                    for c in range(NCH)]
            hbuf = [[spool.tile([128, CH], BF16, tag=f"h{c}{p}", name=f"h{c}{p}") for p in range(2)]
                    for c in range(NCH)]

            ps = [ppool.tile([128, 2048], F32, tag=f"ps{c}", name=f"ps{c}") for c in range(NCH)]

            MM = nc.tensor.matmul

            # ---- precompute: gc = Wz@z + We@enc + b  (per gate tile) ----
            for c in range(NCH):
                cs = slice(c * CH, (c + 1) * CH)
                for g in range(4):
                    gs = slice(g * 128, (g + 1) * 128)
                    pslice = ps[c][:, g * 512:(g + 1) * 512]
                    MM(pslice, WzT[:, gs], zT[:, cs], start=True, stop=False)
                    MM(pslice, WeT0[:, gs], eT0[:, cs], start=False, stop=False)
                    MM(pslice, WeT1[:, gs], eT1[:, cs], start=False, stop=True)
                    nc.scalar.activation(gc[:, g * B + c * CH: g * B + (c + 1) * CH],
                                         pslice, AF.Identity, bias=bias[:, g:g + 1])

            # ---- precompute: u0, h0, c0 ----
            for c in range(NCH):
                cs = slice(c * CH, (c + 1) * CH)
                pslice = ps[c][:, 0:512]
                MM(pslice, Wu0T[:], xT[:, cs], start=True, stop=True)
                nc.scalar.activation(u0[:, cs], pslice, AF.Identity,
                                     bias=bias[:, 4:5])
                nc.vector.tensor_copy(out=u0b[:, cs], in_=u0[:, cs])
            for c in range(NCH):
                cs = slice(c * CH, (c + 1) * CH)
                for W, dst, bcol in ((Wh["Wh1"], h0[:, cs], 5),
                                     (Wh["Wh2"], pair[c][0][:, 0:CH], 6)):
                    pslice = ps[c][:, 512:1024] if bcol == 5 else ps[c][:, 1024:1536]
                    MM(pslice, W[0][:], zT[:, cs], start=True, stop=False)
                    MM(pslice, W[1][:], u0[:, cs], start=False, stop=False)
                    MM(pslice, W[2][:], eT0[:, cs], start=False, stop=False)
                    MM(pslice, W[3][:], eT1[:, cs], start=False, stop=True)
                    nc.scalar.activation(dst, pslice, AF.Identity,
                                         bias=bias[:, bcol:bcol + 1])

            # ---- the scan ----
            def id_mms(c):
                # inject gc into the psum banks for the next step (start=True
                # resets the bank)
                p = ps[c]
                for g in range(4):
                    MM(p[:, g * 512:(g + 1) * 512], idm[:],
                       gc[:, g * B + c * CH: g * B + (c + 1) * CH],
                       start=True, stop=False)

            def step(t, first, last):
                # one LSTM step. `first`: feed u0/h0 instead of h ping-pong.
                par = t % 2
                for c in range(NCH):
                    cs = slice(c * CH, (c + 1) * CH)
                    p = ps[c]
                    pb = p.bitcast(BF16)  # [128, 4096] bf16 view of the banks

                    def rec_mm(g):
                        gsl = p[:, g * 512:(g + 1) * 512]
                        wsl = slice(g * 128, (g + 1) * 128)
                        if first:
                            MM(gsl, WihyT[:, wsl], u0b[:, cs],
                               start=False, stop=False)
                            MM(gsl, WhhT[:, wsl], h0[:, cs],
                               start=False, stop=True)
                        else:
                            MM(gsl, WcT[:, wsl], hbuf[c][(t - 1) % 2][:],
                               start=False, stop=True)

                    # pointwise LSTM cell. bank order [g|f|i|o] (fp32).
                    # sig outs [sf|si], so; pair cols [c|tg]
                    for g in range(4):
                        rec_mm(g)
                    nc.scalar.activation(pair[c][par][:, CH:2 * CH], p[:, 0:CH],
                                         AF.Tanh)
                    nc.scalar.activation(sfi[c][par][:],
                                         p[:, CH:3 * CH], AF.Sigmoid)
                    nc.scalar.activation(sob[c][par][:],
                                         p[:, 3 * CH:4 * CH], AF.Sigmoid)
                    # banks consumed; prefetch next step's gc injection
                    if not last:
                        id_mms(c)
                    nc.vector.tensor_mul(out=prod[c][:], in0=sfi[c][par][:],
                                         in1=pair[c][par][:])
                    nc.vector.tensor_add(out=pair[c][1 - par][:, 0:CH],
                                         in0=prod[c][:, 0:CH],
                                         in1=prod[c][:, CH:2 * CH])
                    nc.scalar.activation(tcell[c][par][:], pair[c][1 - par][:, 0:CH],
                                         AF.Tanh)
                    nc.vector.tensor_mul(out=hbuf[c][par][:],
                                         in0=sob[c][par][:],
                                         in1=tcell[c][par][:])
                    nc.sync.dma_start(out=y_ap[t % T_out, :, cs],
                                      in_=hbuf[c][par][:])

            if loops == 1:
                for c in range(NCH):
                    id_mms(c)
                for t in range(n_steps):
                    step(t, first=(t == 0), last=(t == n_steps - 1))
            else:
                # timing variant: every step takes the steady-state path;
                # h carry-in for the first step is h0 copied into the odd
                # parity buffer (n_steps must be even so parity wraps).
                assert n_steps % 2 == 0
                for c in range(NCH):
                    cs = slice(c * CH, (c + 1) * CH)
                    nc.vector.tensor_copy(out=hbuf[c][1][:], in_=h0[:, cs])
                    id_mms(c)
                with tc.For_i(0, loops):
                    for t in range(n_steps):
                        step(t, first=False, last=False)

    nc.finalize()
    return nc


def _prep_maps(x, enc, z, W_ih, W_hh, b_ih, b_hh, W_u0, b_u0, W_h1, b_h1, W_h2,
               b_h2):
    """Host-side weight prep + per-core sharding. Returns list of in_maps."""
    f = lambda a: np.ascontiguousarray(a, dtype=np.float32)
    fb = lambda a: np.ascontiguousarray(np.asarray(a, dtype=np.float32).astype(ml_dtypes.bfloat16))
    p = GATE_PERM
    Wc = (W_ih[:, LAT:LAT + OUT] + W_hh)[p]          # [512, 128]
    bias_g = (b_ih + b_hh)[p].reshape(4, 128).T      # [128, 4]
    bias7 = np.concatenate(
        [bias_g, b_u0[:, None], b_h1[:, None], b_h2[:, None]], axis=1)

    parts = {
        "Wu0T": f(W_u0.T),
        "WzT": f(W_ih[p, 0:LAT].T),
        "WeT0": f(W_ih[p, LAT + OUT:LAT + OUT + 128].T),
        "WeT1": f(W_ih[p, LAT + OUT + 128:].T),
        "Wh1z": f(W_h1[:, 0:LAT].T), "Wh1y": f(W_h1[:, LAT:LAT + OUT].T),
        "Wh1e0": f(W_h1[:, LAT + OUT:LAT + OUT + 128].T),
        "Wh1e1": f(W_h1[:, LAT + OUT + 128:].T),
        "Wh2z": f(W_h2[:, 0:LAT].T), "Wh2y": f(W_h2[:, LAT:LAT + OUT].T),
        "Wh2e0": f(W_h2[:, LAT + OUT:LAT + OUT + 128].T),
        "Wh2e1": f(W_h2[:, LAT + OUT + 128:].T),
    }
    bf_parts = {
        "WcT": fb(Wc.T),
        "WihyT": fb(W_ih[p, LAT:LAT + OUT].T),
        "WhhT": fb(W_hh[p].T),
        "idm": fb(np.eye(128)),
    }

    def pack(spec, per_core, src, conv):
        return conv(np.concatenate(
            [per_core[nm] if nm in per_core else src[nm] for nm, _ in spec],
            axis=1, dtype=np.float32))

    pkbf = np.ascontiguousarray(np.concatenate(
        [bf_parts[nm] for nm, _ in PKBF_SPEC], axis=1))

    maps = []
    for core in range(N_CORES):
        rows = slice(core * B, (core + 1) * B)
        pc = {"xT": f(x[rows].T), "zT": f(z[rows].T),
              "eT0": f(enc[rows, 0:128].T), "eT1": f(enc[rows, 128:256].T)}
        maps.append({
            "pk64": pack(PK64_SPEC, pc, parts, f),
            "pke0": pack(PKE0_SPEC, pc, parts, f),
            "pke1": pack(PKE1_SPEC, pc, parts, f),
            "pk48": pack(PK48_SPEC, pc, parts, f),
            "pk128b": pack(PK128B_SPEC, pc, parts, f),
            "pkbf": pkbf,
            "bias7": f(bias7),
        })
    return maps


def run_device(maps, n_steps, T_out):
    key = (n_steps, T_out)
    if key not in _PROGRAM_CACHE:
        _PROGRAM_CACHE[key] = build_program(n_steps, T_out)
    nc = _PROGRAM_CACHE[key]
    return run_bass_kernel_spmd(nc, maps, core_ids=list(range(N_CORES)))


def kernel(x, enc, z, W_ih, W_hh, b_ih, b_hh, W_u0, b_u0, W_h1, b_h1, W_h2, b_h2,
           horizon):
    T = int(horizon)
    maps = _prep_maps(np.asarray(x, np.float32), np.asarray(enc, np.float32),
                      np.asarray(z, np.float32), np.asarray(W_ih, np.float32),
                      np.asarray(W_hh, np.float32), np.asarray(b_ih, np.float32),
                      np.asarray(b_hh, np.float32), np.asarray(W_u0, np.float32),
                      np.asarray(b_u0, np.float32), np.asarray(W_h1, np.float32),
                      np.asarray(b_h1, np.float32), np.asarray(W_h2, np.float32),
                      np.asarray(b_h2, np.float32))
    res = run_device(maps, T, T)
    # device y: [T, OUT, B] bf16 per core -> [B, T, 1, OUT] f32, concat cores
    parts = [np.asarray(r["y"]).astype(np.float32).transpose(2, 0, 1)[:, :, None, :]
             for r in res.results]
    return np.ascontiguousarray(np.concatenate(parts, axis=0), dtype=np.float32)
